# revision 1
# baseline (speedup 1.0000x reference)
"""Bidirectional LSTM over embedded event ids — Trainium2 Bass kernel.

Problem shapes (hardcoded): ids [32,64,256] int32, embed [6000,64],
per-direction LSTM E=H=64, output [32,64,256,128] f32.

Strategy: pure data parallel over the flattened B*S=2048 sequence axis,
256 sequences per core on 8 cores. On-device layout keeps the gate/hidden
dim on SBUF partitions and the sequence batch on the free dim, so the
recurrence z = Wcat.T @ [x_t; h_{t-1}] needs no transposes anywhere:

  rhs slot  [128, 256] f32r : parts 0:64 = x_t^T (DMA'd), 64:128 = h_{t-1}^T
  z PSUM    [128, 512]      : cols 0:256 = [i;f] rows, 256:512 = [g;o] rows
  sigmoid over the whole bank (g-weights pre-scaled by 2 so
  tanh(zg) = 2*sigmoid(2 zg) - 1 comes out of a fused affine-multiply)
  c update + h = o*tanh(c) as [64, 256] elementwise ops on parts 64:128.

h is written once, as float32r, directly into the next step's rhs slot;
the output DMA reads the same bytes. Host side does the embedding gather
(sequential-read layout for the device) and folds gate scaling into the
weights.
"""

import numpy as np

B, S, L, E, H, V = 32, 64, 256, 64, 64, 6000
NCORES = 8
NSEQ = B * S
NC_ = NSEQ // NCORES      # 256 sequences per core
GATES = 4 * H             # 256
KDIM = E + H              # 128

_CACHE = {}


def _build(l_steps, nc_seq, with_bias, prefetch=6, reps=1, gates_bf16=False,
           fc_on="pool", tail_prio=0, sigma_split=False,
           out_dma="sync"):
    import concourse.bacc as bacc
    import concourse.tile as tile
    from concourse import mybir

    dt = mybir.dt
    AF = mybir.ActivationFunctionType
    DIRS = ("f", "b")

    nc = bacc.Bacc("TRN2", num_devices=NCORES, debug=False)
    x_d = nc.dram_tensor("x", (E, l_steps, nc_seq), dt.float32r,
                         kind="ExternalInput")
    xr_d = nc.dram_tensor("xr", (E, l_steps, nc_seq), dt.float32r,
                          kind="ExternalInput")
    z0_d = nc.dram_tensor("z0", (H, nc_seq), dt.float32r,
                          kind="ExternalInput")
    w_d = {d: nc.dram_tensor(f"w_{d}", (KDIM, GATES), dt.float32r,
                             kind="ExternalInput") for d in DIRS}
    bias_d = {}
    if with_bias:
        for d in DIRS:
            bias_d[d] = nc.dram_tensor(f"bias_{d}", (KDIM, 2), dt.float32,
                                       kind="ExternalInput")
    o_d = {d: nc.dram_tensor(f"o_{d}", (H, l_steps, nc_seq), dt.float32r,
                             kind="ExternalOutput") for d in DIRS}


    with tile.TileContext(nc) as tc:
        with (
            tc.tile_pool(name="singles", bufs=1) as singles,
            tc.tile_pool(name="rhs", bufs=prefetch + 3) as rhs_pool,
            tc.tile_pool(name="zs", bufs=3) as zs_pool,
            tc.tile_pool(name="tmp", bufs=3) as tmp_pool,
            tc.tile_pool(name="psum_f", bufs=2, space="PSUM") as psum_f,
            tc.tile_pool(name="psum_b", bufs=2, space="PSUM") as psum_b,
        ):
            psum_pool = {"f": psum_f, "b": psum_b}
            w_t = {}
            bias_t = {}
            c_t = {}
            tc_t = {}
            for d in DIRS:
                c_t[d] = singles.tile([128, nc_seq], dt.float32,
                                      name=f"c_{d}", tag=f"c_{d}")
                nc.vector.memset(c_t[d][64:128, :], 0.0)
                tc_t[d] = singles.tile([128, nc_seq], dt.float32,
                                       name=f"tcv_{d}", tag=f"tcv_{d}")
            for d in DIRS:
                w_t[d] = singles.tile([KDIM, GATES], dt.float32r,
                                      name=f"w_{d}", tag=f"w_{d}")
                nc.sync.dma_start(out=w_t[d][:, :], in_=w_d[d].ap())
                if with_bias:
                    bias_t[d] = singles.tile([KDIM, 2], dt.float32,
                                             name=f"biast_{d}", tag=f"bias_{d}")
                    nc.sync.dma_start(out=bias_t[d][:, :], in_=bias_d[d].ap())
            rhs_tiles = {d: {} for d in DIRS}

            def new_slot(d, t):
                tl = rhs_pool.tile([128, nc_seq], dt.float32r,
                                   name=f"rhs_{d}", tag=f"rhs_{d}")
                rhs_tiles[d][t] = tl
                if t < l_steps:
                    src_t = x_d if d == "f" else xr_d
                    nc.sync.dma_start(out=tl[0:64, :],
                                      in_=src_t.ap()[:, t, :])
                return tl

            for d in DIRS:
                for tt in range(min(prefetch, l_steps + 1)):
                    new_slot(d, tt)
                nc.sync.dma_start(out=rhs_tiles[d][0][64:128, :],
                                  in_=z0_d.ap())

            # both dirs: blockA=[i;f], blockB=[g';o]; cell state rows
            # 64:128; the only cross-quadrant access is ig's upward write
            # (reads @0:64, writes @64:128), which is HW-verified
            A, B = slice(0, 64), slice(64, 128)
            ROWS = {"f": {"c": B, "f": B, "i": A, "o": B, "g": A},
                    "b": {"c": B, "f": B, "i": A, "o": B, "g": A}}
            zdt = dt.bfloat16 if gates_bf16 else dt.float32

            for rep in range(reps):
              for t in range(l_steps):
                zs_t = {}
                for d in DIRS:
                    r = ROWS[d]
                    if t + prefetch <= l_steps:
                        new_slot(d, t + prefetch)
                    rhs = rhs_tiles[d][t][:, :]
                    z = psum_pool[d].tile([128, 512], dt.float32,
                                          name=f"z_{d}", tag=f"z_{d}")
                    nc.tensor.matmul(z[:, 0:nc_seq], w_t[d][:, 0:128],
                                     rhs, start=True, stop=True)
                    nc.tensor.matmul(z[:, nc_seq:2 * nc_seq],
                                     w_t[d][:, 128:256],
                                     rhs, start=True, stop=True)
                    zs = zs_pool.tile([128, 512], zdt,
                                       name=f"zs_{d}", tag=f"zs_{d}")
                    zs_t[d] = zs
                    if with_bias:
                        nc.scalar.activation(zs[:, 0:nc_seq], z[:, 0:nc_seq],
                                             AF.Sigmoid,
                                             bias=bias_t[d][:, 0:1])
                        nc.scalar.activation(zs[:, nc_seq:2 * nc_seq],
                                             z[:, nc_seq:2 * nc_seq],
                                             AF.Sigmoid,
                                             bias=bias_t[d][:, 1:2])
                    else:
                        nc.scalar.activation(zs[:, :], z[:, :], AF.Sigmoid)
                    # g = tanh(zg) = 2*sig(2 zg) - 1
                    gg = tmp_pool.tile([128, nc_seq], zdt,
                                       name=f"gg_{d}", tag=f"gg_{d}")
                    nc.vector.tensor_scalar(
                        out=gg[r["g"], :],
                        in0=zs[r["g"], nc_seq:2 * nc_seq],
                        scalar1=2.0, scalar2=1.0,
                        op0=mybir.AluOpType.mult,
                        op1=mybir.AluOpType.subtract)
                    t1 = tmp_pool.tile([128, nc_seq], zdt,
                                       name=f"t1_{d}", tag=f"t1_{d}")
                    nc.vector.tensor_mul(t1[r["c"], :], gg[r["g"], :],
                                         zs[r["i"], 0:nc_seq])
                    # t2 = sig(zf) * c
                    t2 = tmp_pool.tile([128, nc_seq], dt.float32,
                                       name=f"t2_{d}", tag=f"t2_{d}")
                    fc_eng = nc.gpsimd if fc_on == "pool" else nc.vector
                    fc_eng.tensor_mul(t2[r["c"], :],
                                      zs[r["f"], 0:nc_seq],
                                      c_t[d][r["c"], :])
                    nc.vector.tensor_add(c_t[d][r["c"], :],
                                         t1[r["c"], :], t2[r["c"], :])
                    # per-dir tanh keeps the two chains decoupled
                    nc.scalar.activation(tc_t[d][r["c"], :],
                                         c_t[d][r["c"], :], AF.Tanh)
                    nxt = rhs_tiles[d][t + 1]
                    nc.vector.tensor_mul(nxt[64:128, :],
                                         zs[r["o"], nc_seq:2 * nc_seq],
                                         tc_t[d][r["c"], :])
                    out_eng = nc.scalar if out_dma == "act" else nc.sync
                    out_eng.dma_start(out=o_d[d].ap()[:, t, :],
                                      in_=nxt[64:128, :])
                    del rhs_tiles[d][t]

    nc.compile()
    return nc


def _get_nc(l_steps, nc_seq, with_bias):
    key = (l_steps, nc_seq, with_bias)
    if key not in _CACHE:
        _CACHE[key] = _build(l_steps, nc_seq, with_bias)
    return _CACHE[key]


def _prep_w(Wk, Wr, b, mirror=False):
    """[128, 256] f32 contiguous: rows = [x-proj; h-proj], g-gate cols
    pre-scaled by 2 (tanh-via-sigmoid). Keras col order is i,f,g,o;
    device blockA/blockB layouts are [i,f | 2g,o], or mirrored
    [f,i | o,2g] for the fwd direction (see ROWS in _build).
    Returns (Wcat, bias[128,2])."""
    Wcat = np.concatenate([np.asarray(Wk), np.asarray(Wr)], axis=0)
    b = np.asarray(b)
    i_, f_, g_, o_ = (Wcat[:, 0:64], Wcat[:, 64:128],
                      2.0 * Wcat[:, 128:192], Wcat[:, 192:256])
    bi, bf, bg, bo = b[0:64], b[64:128], 2.0 * b[128:192], b[192:256]
    if mirror:
        cols = [f_, i_, o_, g_]
        bcols = [np.concatenate([bf, bi]), np.concatenate([bo, bg])]
    else:
        cols = [i_, f_, g_, o_]
        bcols = [np.concatenate([bi, bf]), np.concatenate([bg, bo])]
    Wout = np.ascontiguousarray(np.concatenate(cols, axis=1),
                                dtype=np.float32)
    bias = None
    if np.any(b != 0.0):
        bias = np.ascontiguousarray(np.stack(bcols, axis=1),
                                    dtype=np.float32)
    return Wout, bias


def kernel(ids, embed_table, Wk_f, Wr_f, b_f, Wk_b, Wr_b, b_b):
    from concourse import bass_utils

    ids = np.asarray(ids)
    embed_table = np.asarray(embed_table, dtype=np.float32)
    wf, bias_f = _prep_w(Wk_f, Wr_f, b_f, mirror=False)
    wb, bias_b = _prep_w(Wk_b, Wr_b, b_b, mirror=False)
    with_bias = bias_f is not None or bias_b is not None
    if with_bias:
        if bias_f is None:
            bias_f = np.zeros((KDIM, 2), np.float32)
        if bias_b is None:
            bias_b = np.zeros((KDIM, 2), np.float32)

    nc = _get_nc(L, NC_, with_bias)

    ids2 = ids.reshape(NSEQ, L)
    in_maps = []
    for m in range(NCORES):
        idc = ids2[m * NC_:(m + 1) * NC_]            # [NC_, L]
        xc = embed_table[idc]                        # [NC_, L, E]
        xT = np.ascontiguousarray(xc.transpose(2, 1, 0))  # [E, L, NC_]
        im = {"x": xT, "xr": np.ascontiguousarray(xT[:, ::-1]),
              "w_f": wf, "w_b": wb,
              "z0": np.zeros((H, NC_), np.float32)}
        if with_bias:
            im["bias_f"] = bias_f
            im["bias_b"] = bias_b
        in_maps.append(im)

    res = bass_utils.run_bass_kernel_spmd(nc, in_maps,
                                          core_ids=list(range(NCORES)))

    out = np.empty((NSEQ, L, 2 * H), dtype=np.float32)
    for m in range(NCORES):
        hf = res.results[m]["o_f"]                   # [H, L, NC_]
        hb = res.results[m]["o_b"][:, ::-1, :]       # iteration -> time order
        sl = slice(m * NC_, (m + 1) * NC_)
        out[sl, :, 0:H] = hf.transpose(2, 1, 0)
        out[sl, :, H:2 * H] = hb.transpose(2, 1, 0)
    return out.reshape(B, S, L, 2 * H)

